# revision 8
# baseline (speedup 1.0000x reference)
"""Cross-attention kernel for Trainium2, sharded over 8 NeuronCores.

Problem (hardcoded shapes): B=2, N=4096, M=1024, DIM=1024, H=16, D=64.
  q = rms_norm(x @ Wq.T + bq)        per-head, gamma gq, eps 1e-6
  k = rms_norm(ctx @ Wk.T + bk)      (Wk = first half of Wkv)
  v = ctx @ Wv.T + bv                (Wv = second half of Wkv)
  out = softmax(q k^T / sqrt(D) + mask_bias) @ v
  y = out @ Wo.T + bo

Sharding: tensor-parallel over the 16 heads -> 2 heads per core.
Each core computes q/k/v projections for its 2 heads (column-sharded
Wq/Wkv), full attention for those heads, and a partial output
projection (row-sharded Wo).  The host sums the 8 partial outputs.

Device-side layout notes:
 - Everything lives "transposed" ([dim, token]) so no activation
   transposes are needed: host feeds x^T / ctx^T, projections produce
   q^T/k^T directly, scores are computed as S^T = k^T.T @ q^T, and the
   PV matmul consumes P^T directly as the moving operand.
 - The context mask is folded into V (V_masked = V * mask), and V is
   augmented with the mask as a 65th column, so the softmax
   denominator drops out of the PV matmul for free (row 64).
 - All matmuls run in float32r (1 cycle/row vs 4 for fp32; measured
   ~1.3e-4 max rel error on hw).
"""

import numpy as np

P = 128
B = 2
N = 4096
M = 1024
C = 1024  # DIM == COND_DIM
H = 16
D = 64
HC = 2  # heads per core
CC = C // P  # contraction chunks
NT = N // 512  # query chunks of 512
MT = M // 512  # kv chunks of 512
MC = M // P  # kv chunks of 128
EPS = 1e-6

_CACHE = {}


def _build():
    if "nc" in _CACHE:
        return _CACHE["nc"]

    import concourse.bass as bass  # noqa: F401
    import concourse.tile as tile
    from concourse import bacc, mybir

    f32 = mybir.dt.float32
    f32r = mybir.dt.float32r
    AF = mybir.ActivationFunctionType

    nc = bacc.Bacc("TRN2", target_bir_lowering=False, debug=False, num_devices=8)

    xt_d = nc.dram_tensor("xt", [B, C, N], f32r, kind="ExternalInput").ap()
    ctxt_d = nc.dram_tensor("ctxt", [B, C, M], f32r, kind="ExternalInput").ap()
    wqt_d = nc.dram_tensor("wqt", [C, P], f32r, kind="ExternalInput").ap()
    wkt_d = nc.dram_tensor("wkt", [C, P], f32r, kind="ExternalInput").ap()
    wvt_d = nc.dram_tensor("wvt", [C, P], f32r, kind="ExternalInput").ap()
    wot_d = nc.dram_tensor("wot", [P, C], f32r, kind="ExternalInput").ap()
    bq_d = nc.dram_tensor("bq", [P, 1], f32, kind="ExternalInput").ap()
    bk_d = nc.dram_tensor("bk", [P, 1], f32, kind="ExternalInput").ap()
    bv_d = nc.dram_tensor("bv", [P, 1], f32, kind="ExternalInput").ap()
    gq_d = nc.dram_tensor("gq", [HC, P], f32r, kind="ExternalInput").ap()
    gk_d = nc.dram_tensor("gk", [HC, P], f32r, kind="ExternalInput").ap()
    ind2_d = nc.dram_tensor("ind2", [P, HC], f32r, kind="ExternalInput").ap()
    ones1_d = nc.dram_tensor("ones1", [1, D], f32r, kind="ExternalInput").ap()
    ident_d = nc.dram_tensor("ident", [P, P], f32, kind="ExternalInput").ap()
    mask_d = nc.dram_tensor("maskf", [P, B * MC], f32, kind="ExternalInput").ap()
    y_d = nc.dram_tensor("y", [B, N, C], f32, kind="ExternalOutput").ap()

    with tile.TileContext(nc) as tc:
        with (
            tc.tile_pool(name="consts", bufs=1) as consts,
            tc.tile_pool(name="ctxp", bufs=1) as ctxp,
            tc.tile_pool(name="xp", bufs=2) as xp,
            tc.tile_pool(name="qk", bufs=1) as qk,
            tc.tile_pool(name="work", bufs=2) as work,
            tc.tile_pool(name="ptp", bufs=3) as ptp,
            tc.tile_pool(name="ps", bufs=1, space="PSUM") as ps,
        ):
            wq_sb = consts.tile([P, CC, P], f32r)
            nc.sync.dma_start(wq_sb[:], wqt_d.rearrange("(o p) m -> p o m", p=P))
            wk_sb = consts.tile([P, CC, P], f32r)
            nc.sync.dma_start(wk_sb[:], wkt_d.rearrange("(o p) m -> p o m", p=P))
            wv_sb = consts.tile([P, CC, P], f32r)
            nc.sync.dma_start(wv_sb[:], wvt_d.rearrange("(o p) m -> p o m", p=P))
            wo_sb = consts.tile([P, C], f32r)
            nc.sync.dma_start(wo_sb[:], wot_d[:])
            bq_sb = consts.tile([P, 1], f32)
            nc.sync.dma_start(bq_sb[:], bq_d[:])
            bk_sb = consts.tile([P, 1], f32)
            nc.sync.dma_start(bk_sb[:], bk_d[:])
            bv_sb = consts.tile([P, 1], f32)
            nc.sync.dma_start(bv_sb[:], bv_d[:])
            gq_sb = consts.tile([HC, P], f32r)
            nc.sync.dma_start(gq_sb[:], gq_d[:])
            gk_sb = consts.tile([HC, P], f32r)
            nc.sync.dma_start(gk_sb[:], gk_d[:])
            ind2_sb = consts.tile([P, HC], f32r)
            nc.sync.dma_start(ind2_sb[:], ind2_d[:])
            ones1_sb = consts.tile([1, D], f32r)
            nc.sync.dma_start(ones1_sb[:], ones1_d[:])
            ident_sb = consts.tile([P, P], f32)
            nc.sync.dma_start(ident_sb[:], ident_d[:])
            mask_sb = consts.tile([P, B * MC], f32)
            nc.sync.dma_start(mask_sb[:], mask_d[:])
            eps_sb = consts.tile([HC, 1], f32)
            nc.vector.memset(eps_sb[:], EPS)

            def rms_norm_chunk(psrc, bias, gind, dst, tagp):
                """psrc: [P, 512] psum of raw (pre-bias) projections for 2
                heads stacked [64|64]; writes normalized f32r into dst."""
                raw = work.tile([P, 512], f32, tag="raw")
                nc.scalar.activation(raw[:], psrc[:], AF.Identity, bias=bias)
                sq = work.tile([P, 512], f32r, tag="sq")
                nc.vector.tensor_mul(sq[:], raw[:], raw[:])
                ss = ps.tile([HC, 512], f32, tag="B2", bufs=2)
                nc.tensor.matmul(ss[:], ind2_sb[:], sq[:], start=True, stop=True)
                srt = work.tile([HC, 512], f32, tag="srt")
                nc.scalar.activation(
                    srt[:], ss[:], AF.Sqrt, scale=1.0 / D, bias=eps_sb[:]
                )
                rstd = work.tile([HC, 512], f32r, tag="rstd")
                with nc.allow_low_precision(reason="f32r rstd"):
                    nc.vector.reciprocal(rstd[:], srt[:])
                bc = ps.tile([P, 512], f32, tag="O", bufs=3)
                nc.tensor.matmul(bc[:], gind[:], rstd[:], start=True, stop=True)
                nc.vector.tensor_mul(dst, raw[:], bc[:])

            for b in range(B):
                # ---- KV phase ----
                ctx_sb = ctxp.tile([P, CC, M], f32r, tag="ctx")
                nc.sync.dma_start(
                    ctx_sb[:], ctxt_d[b].rearrange("(o p) m -> p o m", p=P)
                )
                ktn = qk.tile([P, M], f32r, tag="ktn")
                for mt in range(MT):
                    ps_k = ps.tile([P, 512], f32, tag="A", bufs=3)
                    for cc in range(CC):
                        nc.tensor.matmul(
                            ps_k[:],
                            wk_sb[:, cc],
                            ctx_sb[:, cc, mt * 512 : (mt + 1) * 512],
                            start=(cc == 0),
                            stop=(cc == CC - 1),
                        )
                    rms_norm_chunk(
                        ps_k, bk_sb, gk_sb, ktn[:, mt * 512 : (mt + 1) * 512], "k"
                    )
                vaug = qk.tile([P, MC, 2 * (D + 1)], f32r, tag="vaug")
                for mt in range(MT):
                    ps_v = ps.tile([P, 512], f32, tag="A", bufs=3)
                    for cc in range(CC):
                        nc.tensor.matmul(
                            ps_v[:],
                            wv_sb[:, cc],
                            ctx_sb[:, cc, mt * 512 : (mt + 1) * 512],
                            start=(cc == 0),
                            stop=(cc == CC - 1),
                        )
                    vt_sb = work.tile([P, 512], f32, tag="vt")
                    nc.scalar.activation(vt_sb[:], ps_v[:], AF.Identity, bias=bv_sb)
                    for i in range(4):
                        mc = mt * 4 + i
                        ps_t = ps.tile([P, P], f32, tag="O", bufs=3)
                        nc.tensor.transpose(
                            ps_t[:], vt_sb[:, i * P : (i + 1) * P], ident_sb[:]
                        )
                        mcol = mask_sb[:, b * MC + mc : b * MC + mc + 1]
                        nc.vector.tensor_mul(
                            vaug[:, mc, 0:D], ps_t[:, 0:D], mcol.to_broadcast((P, D))
                        )
                        nc.vector.tensor_mul(
                            vaug[:, mc, D + 1 : 2 * D + 1],
                            ps_t[:, D : 2 * D],
                            mcol.to_broadcast((P, D)),
                        )
                        nc.vector.tensor_copy(vaug[:, mc, D : D + 1], mcol)
                        nc.vector.tensor_copy(vaug[:, mc, 2 * D + 1 : 2 * D + 2], mcol)

                # ---- Q proj + attention + out proj, pipelined over n ----
                qtn = qk.tile([P, N], f32r, tag="qtn")
                outtn = qk.tile([P, N], f32r, tag="outtn")
                for nt in range(NT):
                    nsl = slice(nt * 512, (nt + 1) * 512)
                    xt_sb = xp.tile([P, CC, 512], f32r, tag="xt")
                    nc.sync.dma_start(
                        xt_sb[:],
                        xt_d[b].rearrange("(o p) n -> p o n", p=P)[:, :, nsl],
                    )
                    ps_q = ps.tile([P, 512], f32, tag="A", bufs=3)
                    for cc in range(CC):
                        nc.tensor.matmul(
                            ps_q[:],
                            wq_sb[:, cc],
                            xt_sb[:, cc],
                            start=(cc == 0),
                            stop=(cc == CC - 1),
                        )
                    rms_norm_chunk(ps_q, bq_sb, gq_sb, qtn[:, nsl], "q")

                    # attention for this n-chunk, both heads interleaved
                    ps_o = [
                        ps.tile([D + 1, 512], f32, tag="O", bufs=3, name=f"ps_o{h}")
                        for h in range(2)
                    ]
                    for mc in range(MC):
                        msl = slice(mc * P, (mc + 1) * P)
                        pts = []
                        for h in range(2):
                            hsl = slice(D * h, D * (h + 1))
                            ps_s = ps.tile([P, 512], f32, tag="A", bufs=3)
                            nc.tensor.matmul(
                                ps_s[:],
                                ktn[hsl, msl],
                                qtn[hsl, nsl],
                                start=True,
                                stop=True,
                            )
                            pt = ptp.tile([P, 512], f32r, tag="pt")
                            nc.scalar.activation(pt[:], ps_s[:], AF.Exp)
                            pts.append(pt)
                        for h in range(2):
                            nc.tensor.matmul(
                                ps_o[h][:],
                                vaug[:, mc, h * (D + 1) : (h + 1) * (D + 1)],
                                pts[h][:],
                                start=(mc == 0),
                                stop=(mc == MC - 1),
                            )
                    for h in range(2):
                        recrow = work.tile([1, 512], f32r, tag="recrow")
                        with nc.allow_low_precision(reason="f32r recip"):
                            nc.vector.reciprocal(recrow[:], ps_o[h][D : D + 1, :])
                        bc2 = ps.tile([D, 512], f32, tag="B2", bufs=2)
                        nc.tensor.matmul(
                            bc2[:], ones1_sb[:], recrow[:], start=True, stop=True
                        )
                        oraw = work.tile([D, 512], f32, tag="oraw")
                        nc.scalar.activation(oraw[:], ps_o[h][0:D, :], AF.Copy)
                        nc.vector.tensor_mul(
                            outtn[D * h : D * (h + 1), nsl], oraw[:], bc2[:]
                        )

                    # out-projection for the 4 t-chunks of this n-chunk
                    for i in range(4):
                        tc_ = nt * 4 + i
                        y_sb = work.tile([P, C], f32, tag="ysb")
                        for ec in range(2):
                            ps_y = ps.tile([P, 512], f32, tag="A", bufs=3)
                            nc.tensor.matmul(
                                ps_y[:],
                                outtn[:, tc_ * P : (tc_ + 1) * P],
                                wo_sb[:, ec * 512 : (ec + 1) * 512],
                                start=True,
                                stop=True,
                            )
                            esl = slice(ec * 512, (ec + 1) * 512)
                            if ec == 0:
                                nc.scalar.activation(y_sb[:, esl], ps_y[:], AF.Copy)
                            else:
                                nc.vector.tensor_copy(y_sb[:, esl], ps_y[:])
                        nc.sync.dma_start(y_d[b, tc_ * P : (tc_ + 1) * P, :], y_sb[:])

    nc.compile()
    _CACHE["nc"] = nc
    return nc


def _make_in_maps(x, context, context_mask, Wq, bq, Wkv, bkv, gq, gk, Wo, bo):
    f32 = np.float32
    xt = np.ascontiguousarray(np.transpose(x, (0, 2, 1)), dtype=f32)
    ctxt = np.ascontiguousarray(np.transpose(context, (0, 2, 1)), dtype=f32)
    # maskf[p, b*MC + mc] = mask[b, mc*128 + p]
    maskf = np.ascontiguousarray(
        np.transpose(
            np.asarray(context_mask, dtype=f32).reshape(B, MC, P), (2, 0, 1)
        ).reshape(P, B * MC)
    )
    ident = np.eye(P, dtype=f32)
    ones1 = np.ones((1, D), dtype=f32)
    ind2 = np.zeros((P, HC), dtype=f32)
    for h in range(HC):
        ind2[D * h : D * (h + 1), h] = 1.0

    in_maps = []
    for c in range(8):
        hs = slice(P * c, P * (c + 1))
        gq_c = np.zeros((HC, P), dtype=f32)
        gk_c = np.zeros((HC, P), dtype=f32)
        for h in range(HC):
            gq_c[h, D * h : D * (h + 1)] = gq[HC * c + h] * (1.0 / np.sqrt(D))
            gk_c[h, D * h : D * (h + 1)] = gk[HC * c + h]
        in_maps.append(
            {
                "xt": xt,
                "ctxt": ctxt,
                "wqt": np.ascontiguousarray(Wq[hs].T, dtype=f32),
                "wkt": np.ascontiguousarray(Wkv[hs].T, dtype=f32),
                "wvt": np.ascontiguousarray(Wkv[C + P * c : C + P * (c + 1)].T, dtype=f32),
                "wot": np.ascontiguousarray(Wo[:, hs].T, dtype=f32),
                "bq": np.asarray(bq[hs], dtype=f32).reshape(P, 1),
                "bk": np.asarray(bkv[hs], dtype=f32).reshape(P, 1),
                "bv": np.asarray(bkv[C + P * c : C + P * (c + 1)], dtype=f32).reshape(P, 1),
                "gq": gq_c,
                "gk": gk_c,
                "ind2": ind2,
                "ones1": ones1,
                "ident": ident,
                "maskf": maskf,
            }
        )
    return in_maps


def _run(in_maps, **spmd_kwargs):
    from concourse import bass_utils

    nc = _build()
    return bass_utils.run_bass_kernel_spmd(
        nc, in_maps, core_ids=list(range(8)), **spmd_kwargs
    )


def kernel(x, context, context_mask, Wq, bq, Wkv, bkv, gq, gk, Wo, bo):
    in_maps = _make_in_maps(
        x, context, context_mask, Wq, bq, Wkv, bkv, gq, gk, Wo, bo
    )
    res = _run(in_maps)
    y = np.zeros((B, N, C), dtype=np.float64)
    for c in range(8):
        y += res.results[c]["y"]
    y += np.asarray(bo, dtype=np.float64)
    return y.astype(np.float32)


# revision 11
# speedup vs baseline: 1.2560x; 1.2560x over previous
"""Cross-attention kernel for Trainium2, sharded over 8 NeuronCores.

Problem (hardcoded shapes): B=2, N=4096, M=1024, DIM=1024, H=16, D=64.
  q = rms_norm(x @ Wq.T + bq)        per-head, gamma gq, eps 1e-6
  k = rms_norm(ctx @ Wk.T + bk)      (Wk = first half of Wkv)
  v = ctx @ Wv.T + bv                (Wv = second half of Wkv)
  out = softmax(q k^T / sqrt(D) + mask_bias) @ v
  y = out @ Wo.T + bo

Sharding: tensor-parallel over the 16 heads -> 2 heads per core.
Each core computes q/k/v projections for its 2 heads (column-sharded
Wq/Wkv), full attention for those heads, and a partial output
projection (row-sharded Wo).  The host sums the 8 partial outputs.

Device-side layout notes:
 - Everything lives "transposed" ([dim, token]) so no activation
   transposes are needed: host feeds x^T / ctx^T, projections produce
   q^T/k^T directly, scores are computed as S^T = k^T.T @ q^T, and the
   PV matmul consumes P^T directly as the moving operand.
 - The context mask is folded into V (V_masked = V * mask), and V is
   augmented with the mask as a 65th column, so the softmax
   denominator drops out of the PV matmul for free (row 64).
 - All matmuls run in float32r (1 cycle/row vs 4 for fp32; measured
   ~1.3e-4 max rel error on hw).
 - Phases are kept separate (proj / attention / out-proj) so ACT only
   alternates its table function twice per batch element, and the PE
   stream stays dense (HAM stays un-throttled).
"""

import numpy as np

P = 128
B = 2
N = 4096
M = 1024
C = 1024  # DIM == COND_DIM
H = 16
D = 64
HC = 2  # heads per core
CC = C // P  # contraction chunks
NT = N // 512  # query chunks of 512
MT = M // 512  # kv chunks of 512
MC = M // P  # kv chunks of 128
EPS = 1e-6

_CACHE = {}


def _build():
    if "nc" in _CACHE:
        return _CACHE["nc"]

    import concourse.bass as bass  # noqa: F401
    import concourse.tile as tile
    from concourse import bacc, mybir

    f32 = mybir.dt.float32
    f32r = mybir.dt.float32r
    AF = mybir.ActivationFunctionType
    MUL = mybir.AluOpType.mult

    nc = bacc.Bacc("TRN2", target_bir_lowering=False, debug=False, num_devices=8)

    xt_d = nc.dram_tensor("xt", [B, C, N], f32r, kind="ExternalInput").ap()
    ctxt_d = nc.dram_tensor("ctxt", [B, C, M], f32r, kind="ExternalInput").ap()
    wqt_d = nc.dram_tensor("wqt", [C, P], f32r, kind="ExternalInput").ap()
    wkt_d = nc.dram_tensor("wkt", [C, P], f32r, kind="ExternalInput").ap()
    wvt_d = nc.dram_tensor("wvt", [C, P], f32r, kind="ExternalInput").ap()
    wot_d = nc.dram_tensor("wot", [P, C], f32r, kind="ExternalInput").ap()
    bq_d = nc.dram_tensor("bq", [P, 1], f32, kind="ExternalInput").ap()
    bk_d = nc.dram_tensor("bk", [P, 1], f32, kind="ExternalInput").ap()
    bv_d = nc.dram_tensor("bv", [P, 1], f32, kind="ExternalInput").ap()
    gq_d = nc.dram_tensor("gq", [HC, P], f32r, kind="ExternalInput").ap()
    gk_d = nc.dram_tensor("gk", [HC, P], f32r, kind="ExternalInput").ap()
    ind2_d = nc.dram_tensor("ind2", [P, HC], f32r, kind="ExternalInput").ap()
    ident_d = nc.dram_tensor("ident", [P, P], f32, kind="ExternalInput").ap()
    mask_d = nc.dram_tensor("maskf", [P, B * MC], f32, kind="ExternalInput").ap()
    y_d = nc.dram_tensor("y", [B, N, C], f32, kind="ExternalOutput").ap()

    with tile.TileContext(nc) as tc:
        with (
            tc.tile_pool(name="consts", bufs=1) as consts,
            tc.tile_pool(name="ctxp", bufs=1) as ctxp,
            tc.tile_pool(name="xp", bufs=2) as xp,
            tc.tile_pool(name="qk", bufs=1) as qk,
            tc.tile_pool(name="work", bufs=2) as work,
            tc.tile_pool(name="ptp", bufs=3) as ptp,
            tc.tile_pool(name="ps", bufs=1, space="PSUM") as ps,
        ):
            wq_sb = consts.tile([P, CC, P], f32r)
            nc.sync.dma_start(wq_sb[:], wqt_d.rearrange("(o p) m -> p o m", p=P))
            wk_sb = consts.tile([P, CC, P], f32r)
            nc.sync.dma_start(wk_sb[:], wkt_d.rearrange("(o p) m -> p o m", p=P))
            wv_sb = consts.tile([P, CC, P], f32r)
            nc.sync.dma_start(wv_sb[:], wvt_d.rearrange("(o p) m -> p o m", p=P))
            wo_sb = consts.tile([P, C], f32r)
            nc.sync.dma_start(wo_sb[:], wot_d[:])
            bq_sb = consts.tile([P, 1], f32)
            nc.sync.dma_start(bq_sb[:], bq_d[:])
            bk_sb = consts.tile([P, 1], f32)
            nc.sync.dma_start(bk_sb[:], bk_d[:])
            bv_sb = consts.tile([P, 1], f32)
            nc.sync.dma_start(bv_sb[:], bv_d[:])
            gq_sb = consts.tile([HC, P], f32r)
            nc.sync.dma_start(gq_sb[:], gq_d[:])
            gk_sb = consts.tile([HC, P], f32r)
            nc.sync.dma_start(gk_sb[:], gk_d[:])
            ind2_sb = consts.tile([P, HC], f32r)
            nc.sync.dma_start(ind2_sb[:], ind2_d[:])
            ident_sb = consts.tile([P, P], f32)
            nc.sync.dma_start(ident_sb[:], ident_d[:])
            mask_sb = consts.tile([P, B * MC], f32)
            nc.sync.dma_start(mask_sb[:], mask_d[:])
            eps_sb = consts.tile([HC, 1], f32)
            nc.vector.memset(eps_sb[:], EPS)

            def rms_norm_chunk(psrc, bias, gind, dst):
                """psrc: [P, 512] psum of raw (pre-bias) projections for 2
                heads stacked [64|64]; writes normalized f32r into dst."""
                raw = work.tile([P, 512], f32, tag="raw")
                nc.vector.tensor_scalar_add(raw[:], psrc[:], bias)
                sq = work.tile([P, 512], f32r, tag="sq")
                nc.vector.tensor_mul(sq[:], raw[:], raw[:])
                ss = ps.tile([HC, 512], f32, tag="B2", bufs=2)
                nc.tensor.matmul(ss[:], ind2_sb[:], sq[:], start=True, stop=True)
                srt = work.tile([HC, 512], f32, tag="srt")
                nc.scalar.activation(
                    srt[:], ss[:], AF.Sqrt, scale=1.0 / D, bias=eps_sb[:]
                )
                rstd_f = work.tile([HC, 512], f32, tag="rstd_f")
                nc.vector.reciprocal_approx_fast(out=rstd_f[:], in_=srt[:])
                rstd = work.tile([HC, 512], f32r, tag="rstd")
                nc.vector.tensor_copy(rstd[:], rstd_f[:])
                bc = ps.tile([P, 512], f32, tag="O", bufs=3)
                nc.tensor.matmul(bc[:], gind[:], rstd[:], start=True, stop=True)
                nc.vector.tensor_mul(dst, raw[:], bc[:])

            for b in range(B):
                # ---- KV phase ----
                ctx_sb = ctxp.tile([P, CC, M], f32r, tag="ctx")
                nc.sync.dma_start(
                    ctx_sb[:], ctxt_d[b].rearrange("(o p) m -> p o m", p=P)
                )
                ktn = qk.tile([P, M], f32r, tag="ktn", bufs=2)
                for mt in range(MT):
                    ps_k = ps.tile([P, 512], f32, tag="A", bufs=3)
                    for cc in range(CC):
                        nc.tensor.matmul(
                            ps_k[:],
                            wk_sb[:, cc],
                            ctx_sb[:, cc, mt * 512 : (mt + 1) * 512],
                            start=(cc == 0),
                            stop=(cc == CC - 1),
                        )
                    rms_norm_chunk(ps_k, bk_sb, gk_sb, ktn[:, mt * 512 : (mt + 1) * 512])
                vaug = qk.tile([P, MC, 2 * (D + 1)], f32r, tag="vaug", bufs=2)
                for mt in range(MT):
                    ps_v = ps.tile([P, 512], f32, tag="A", bufs=3)
                    for cc in range(CC):
                        nc.tensor.matmul(
                            ps_v[:],
                            wv_sb[:, cc],
                            ctx_sb[:, cc, mt * 512 : (mt + 1) * 512],
                            start=(cc == 0),
                            stop=(cc == CC - 1),
                        )
                    vt_sb = work.tile([P, 512], f32, tag="vt")
                    nc.vector.tensor_scalar_add(vt_sb[:], ps_v[:], bv_sb)
                    for i in range(4):
                        mc = mt * 4 + i
                        ps_t = ps.tile([P, P], f32, tag="O", bufs=3)
                        nc.tensor.transpose(
                            ps_t[:], vt_sb[:, i * P : (i + 1) * P], ident_sb[:]
                        )
                        mcol = mask_sb[:, b * MC + mc : b * MC + mc + 1]
                        nc.vector.tensor_mul(
                            vaug[:, mc, 0:D], ps_t[:, 0:D], mcol.to_broadcast((P, D))
                        )
                        nc.vector.tensor_copy(vaug[:, mc, D : D + 1], mcol)
                        nc.vector.tensor_mul(
                            vaug[:, mc, D + 1 : 2 * D + 1],
                            ps_t[:, D : 2 * D],
                            mcol.to_broadcast((P, D)),
                        )
                        nc.vector.tensor_copy(vaug[:, mc, 2 * D + 1 : 2 * D + 2], mcol)

                # ---- Q projection phase (norms lag one chunk behind) ----
                qtn = qk.tile([P, N], f32r, tag="qtn")
                pending = []
                for nt in range(NT):
                    nsl = slice(nt * 512, (nt + 1) * 512)
                    xt_sb = xp.tile([P, CC, 512], f32r, tag="xt")
                    nc.sync.dma_start(
                        xt_sb[:],
                        xt_d[b].rearrange("(o p) n -> p o n", p=P)[:, :, nsl],
                    )
                    ps_q = ps.tile([P, 512], f32, tag="A", bufs=3)
                    for cc in range(CC):
                        nc.tensor.matmul(
                            ps_q[:],
                            wq_sb[:, cc],
                            xt_sb[:, cc],
                            start=(cc == 0),
                            stop=(cc == CC - 1),
                        )
                    pending.append((ps_q, nsl))
                    if len(pending) >= 2:
                        pq, pn = pending.pop(0)
                        rms_norm_chunk(pq, bq_sb, gq_sb, qtn[:, pn])
                for pq, pn in pending:
                    rms_norm_chunk(pq, bq_sb, gq_sb, qtn[:, pn])

                # ---- Attention phase ----
                outtn = qk.tile([P, N], f32r, tag="outtn")
                for nt in range(NT):
                    nsl = slice(nt * 512, (nt + 1) * 512)
                    ps_o = [
                        ps.tile([D + 1, 512], f32, tag="O", bufs=3, name=f"ps_o{h}")
                        for h in range(2)
                    ]
                    for mc in range(MC):
                        msl = slice(mc * P, (mc + 1) * P)
                        pts = []
                        for h in range(2):
                            hsl = slice(D * h, D * (h + 1))
                            ps_s = ps.tile([P, 512], f32, tag="A", bufs=3)
                            nc.tensor.matmul(
                                ps_s[:],
                                ktn[hsl, msl],
                                qtn[hsl, nsl],
                                start=True,
                                stop=True,
                            )
                            pt = ptp.tile([P, 512], f32r, tag="pt")
                            nc.scalar.activation(pt[:], ps_s[:], AF.Exp)
                            pts.append(pt)
                        for h in range(2):
                            nc.tensor.matmul(
                                ps_o[h][:],
                                vaug[:, mc, h * (D + 1) : (h + 1) * (D + 1)],
                                pts[h][:],
                                start=(mc == 0),
                                stop=(mc == MC - 1),
                            )
                    for h in range(2):
                        denrow = work.tile([1, 512], f32, tag="denrow")
                        nc.vector.tensor_copy(denrow[:], ps_o[h][D : D + 1, :])
                        recrow = work.tile([1, 512], f32, tag="recrow")
                        nc.vector.reciprocal_approx_fast(out=recrow[:], in_=denrow[:])
                        bcn = work.tile([D, 512], f32, tag="bcn")
                        nc.gpsimd.partition_broadcast(bcn[:], recrow[:])
                        nc.vector.scalar_tensor_tensor(
                            out=outtn[D * h : D * (h + 1), nsl],
                            in0=ps_o[h][0:D, :],
                            scalar=1.0,
                            in1=bcn[:],
                            op0=MUL,
                            op1=MUL,
                        )

                # ---- Output projection phase ----
                for tc_ in range(N // P):
                    y_sb = work.tile([P, C], f32, tag="ysb")
                    for ec in range(2):
                        ps_y = ps.tile([P, 512], f32, tag="A", bufs=3)
                        nc.tensor.matmul(
                            ps_y[:],
                            outtn[:, tc_ * P : (tc_ + 1) * P],
                            wo_sb[:, ec * 512 : (ec + 1) * 512],
                            start=True,
                            stop=True,
                        )
                        esl = slice(ec * 512, (ec + 1) * 512)
                        if ec == 0:
                            nc.scalar.activation(y_sb[:, esl], ps_y[:], AF.Copy)
                        else:
                            nc.vector.tensor_copy(y_sb[:, esl], ps_y[:])
                    nc.sync.dma_start(y_d[b, tc_ * P : (tc_ + 1) * P, :], y_sb[:])

    nc.compile()
    _CACHE["nc"] = nc
    return nc


def _make_in_maps(x, context, context_mask, Wq, bq, Wkv, bkv, gq, gk, Wo, bo):
    f32 = np.float32
    xt = np.ascontiguousarray(np.transpose(x, (0, 2, 1)), dtype=f32)
    ctxt = np.ascontiguousarray(np.transpose(context, (0, 2, 1)), dtype=f32)
    # maskf[p, b*MC + mc] = mask[b, mc*128 + p]
    maskf = np.ascontiguousarray(
        np.transpose(
            np.asarray(context_mask, dtype=f32).reshape(B, MC, P), (2, 0, 1)
        ).reshape(P, B * MC)
    )
    ident = np.eye(P, dtype=f32)
    ind2 = np.zeros((P, HC), dtype=f32)
    for h in range(HC):
        ind2[D * h : D * (h + 1), h] = 1.0

    in_maps = []
    for c in range(8):
        hs = slice(P * c, P * (c + 1))
        gq_c = np.zeros((HC, P), dtype=f32)
        gk_c = np.zeros((HC, P), dtype=f32)
        for h in range(HC):
            gq_c[h, D * h : D * (h + 1)] = gq[HC * c + h] * (1.0 / np.sqrt(D))
            gk_c[h, D * h : D * (h + 1)] = gk[HC * c + h]
        in_maps.append(
            {
                "xt": xt,
                "ctxt": ctxt,
                "wqt": np.ascontiguousarray(Wq[hs].T, dtype=f32),
                "wkt": np.ascontiguousarray(Wkv[hs].T, dtype=f32),
                "wvt": np.ascontiguousarray(Wkv[C + P * c : C + P * (c + 1)].T, dtype=f32),
                "wot": np.ascontiguousarray(Wo[:, hs].T, dtype=f32),
                "bq": np.asarray(bq[hs], dtype=f32).reshape(P, 1),
                "bk": np.asarray(bkv[hs], dtype=f32).reshape(P, 1),
                "bv": np.asarray(bkv[C + P * c : C + P * (c + 1)], dtype=f32).reshape(P, 1),
                "gq": gq_c,
                "gk": gk_c,
                "ind2": ind2,
                "ident": ident,
                "maskf": maskf,
            }
        )
    return in_maps


def _run(in_maps, **spmd_kwargs):
    from concourse import bass_utils

    nc = _build()
    return bass_utils.run_bass_kernel_spmd(
        nc, in_maps, core_ids=list(range(8)), **spmd_kwargs
    )


def kernel(x, context, context_mask, Wq, bq, Wkv, bkv, gq, gk, Wo, bo):
    in_maps = _make_in_maps(
        x, context, context_mask, Wq, bq, Wkv, bkv, gq, gk, Wo, bo
    )
    res = _run(in_maps)
    y = np.zeros((B, N, C), dtype=np.float64)
    for c in range(8):
        y += res.results[c]["y"]
    y += np.asarray(bo, dtype=np.float64)
    return y.astype(np.float32)


# revision 15
# speedup vs baseline: 1.3478x; 1.0731x over previous
"""Cross-attention kernel for Trainium2, sharded over 8 NeuronCores.

Problem (hardcoded shapes): B=2, N=4096, M=1024, DIM=1024, H=16, D=64.
  q = rms_norm(x @ Wq.T + bq)        per-head, gamma gq, eps 1e-6
  k = rms_norm(ctx @ Wk.T + bk)      (Wk = first half of Wkv)
  v = ctx @ Wv.T + bv                (Wv = second half of Wkv)
  out = softmax(q k^T / sqrt(D) + mask_bias) @ v
  y = out @ Wo.T + bo

Sharding: tensor-parallel over the 16 heads -> 2 heads per core.
Each core computes q/k/v projections for its 2 heads (column-sharded
Wq/Wkv), full attention for those heads, and a partial output
projection (row-sharded Wo).  The host sums the 8 partial outputs.

Device-side layout notes:
 - Everything lives "transposed" ([dim, token]) so no activation
   transposes are needed: host feeds x^T / ctx^T, projections produce
   q^T/k^T directly, scores are computed as S^T = k^T.T @ q^T, and the
   PV matmul consumes P^T directly as the moving operand.
 - The context mask is folded into V (V_masked = V * mask), and V is
   augmented with the mask as a 65th column, so the softmax
   denominator drops out of the PV matmul for free (row 64).
 - All matmuls run in float32r (1 cycle/row vs 4 for fp32; measured
   ~1.3e-4 max rel error on hw).
 - PSUM tiles are 2 banks wide ([128, 1024]); matmuls write 512-wide
   halves, elementwise consumers read the full 1024 in one op.
 - Phases are kept separate (proj / attention / out-proj) so ACT only
   alternates its table function twice per batch element, and the PE
   stream stays dense (HAM stays un-throttled).
"""

import numpy as np

P = 128
B = 2
N = 4096
M = 1024
C = 1024  # DIM == COND_DIM
H = 16
D = 64
HC = 2  # heads per core
CC = C // P  # contraction chunks
NT = N // 1024  # query chunks of 1024
MC = M // P  # kv chunks of 128
EPS = 1e-6

_CACHE = {}


def _build():
    if "nc" in _CACHE:
        return _CACHE["nc"]

    import concourse.bass as bass  # noqa: F401
    import concourse.tile as tile
    from concourse import bacc, mybir

    f32 = mybir.dt.float32
    f32r = mybir.dt.float32r
    AF = mybir.ActivationFunctionType
    MUL = mybir.AluOpType.mult

    nc = bacc.Bacc("TRN2", target_bir_lowering=False, debug=False, num_devices=8)

    xt_d = nc.dram_tensor("xt", [B, C, N], f32r, kind="ExternalInput").ap()
    ctxt_d = nc.dram_tensor("ctxt", [B, C, M], f32r, kind="ExternalInput").ap()
    wqt_d = nc.dram_tensor("wqt", [C, P], f32r, kind="ExternalInput").ap()
    wkt_d = nc.dram_tensor("wkt", [C, P], f32r, kind="ExternalInput").ap()
    wvt_d = nc.dram_tensor("wvt", [C, P], f32r, kind="ExternalInput").ap()
    wot_d = nc.dram_tensor("wot", [P, C], f32r, kind="ExternalInput").ap()
    bq_d = nc.dram_tensor("bq", [P, 1], f32, kind="ExternalInput").ap()
    bk_d = nc.dram_tensor("bk", [P, 1], f32, kind="ExternalInput").ap()
    bv_d = nc.dram_tensor("bv", [P, 1], f32, kind="ExternalInput").ap()
    gq_d = nc.dram_tensor("gq", [HC, P], f32r, kind="ExternalInput").ap()
    gk_d = nc.dram_tensor("gk", [HC, P], f32r, kind="ExternalInput").ap()
    ind2_d = nc.dram_tensor("ind2", [P, HC], f32r, kind="ExternalInput").ap()
    ident_d = nc.dram_tensor("ident", [P, P], f32, kind="ExternalInput").ap()
    mask_d = nc.dram_tensor("maskf", [P, B * MC], f32, kind="ExternalInput").ap()
    y_d = nc.dram_tensor("y", [B, N, C], f32, kind="ExternalOutput").ap()

    with tile.TileContext(nc) as tc:
        with (
            tc.tile_pool(name="consts", bufs=1) as consts,
            tc.tile_pool(name="ctxp", bufs=1) as ctxp,
            tc.tile_pool(name="xp", bufs=2) as xp,
            tc.tile_pool(name="qk", bufs=1) as qk,
            tc.tile_pool(name="work", bufs=2) as work,
            tc.tile_pool(name="ptp", bufs=3) as ptp,
            tc.tile_pool(name="ps", bufs=1, space="PSUM") as ps,
        ):
            wq_sb = consts.tile([P, CC, P], f32r)
            nc.sync.dma_start(wq_sb[:], wqt_d.rearrange("(o p) m -> p o m", p=P))
            wk_sb = consts.tile([P, CC, P], f32r)
            nc.sync.dma_start(wk_sb[:], wkt_d.rearrange("(o p) m -> p o m", p=P))
            wv_sb = consts.tile([P, CC, P], f32r)
            nc.sync.dma_start(wv_sb[:], wvt_d.rearrange("(o p) m -> p o m", p=P))
            wo_sb = consts.tile([P, C], f32r)
            nc.sync.dma_start(wo_sb[:], wot_d[:])
            bq_sb = consts.tile([P, 1], f32)
            nc.sync.dma_start(bq_sb[:], bq_d[:])
            bk_sb = consts.tile([P, 1], f32)
            nc.sync.dma_start(bk_sb[:], bk_d[:])
            bv_sb = consts.tile([P, 1], f32)
            nc.sync.dma_start(bv_sb[:], bv_d[:])
            gq_sb = consts.tile([HC, P], f32r)
            nc.sync.dma_start(gq_sb[:], gq_d[:])
            gk_sb = consts.tile([HC, P], f32r)
            nc.sync.dma_start(gk_sb[:], gk_d[:])
            ind2_sb = consts.tile([P, HC], f32r)
            nc.sync.dma_start(ind2_sb[:], ind2_d[:])
            ident_sb = consts.tile([P, P], f32)
            nc.sync.dma_start(ident_sb[:], ident_d[:])
            mask_sb = consts.tile([P, B * MC], f32)
            nc.sync.dma_start(mask_sb[:], mask_d[:])
            eps_sb = consts.tile([HC, 1], f32)
            nc.vector.memset(eps_sb[:], EPS)

            def rms_norm_chunk(psrc, bias, gind, dst, fw):
                """psrc: [P, fw] psum (fw in {512,1024}) of raw projections
                for 2 heads stacked [64|64]; writes normalized f32r to dst."""
                raw = work.tile([P, 1024], f32, tag="raw", name="raw")[:, :fw]
                nc.vector.tensor_scalar_add(raw, psrc, bias)
                sq = work.tile([P, 1024], f32r, tag="sq", name="sq")[:, :fw]
                nc.vector.tensor_mul(sq, raw, raw)
                ss = ps.tile([HC, 1024], f32, tag="A", bufs=3, name="ss")[:, :fw]
                for half in range(fw // 512):
                    hs = slice(half * 512, (half + 1) * 512)
                    nc.tensor.matmul(
                        ss[:, hs], ind2_sb[:], sq[:, hs], start=True, stop=True
                    )
                srt = work.tile([HC, 1024], f32, tag="srt", name="srt", bufs=1)[:, :fw]
                nc.scalar.activation(srt, ss, AF.Sqrt, scale=1.0 / D, bias=eps_sb[:])
                rstd_f = work.tile([HC, 1024], f32, tag="rstd_f", name="rstd_f", bufs=1)[:, :fw]
                nc.vector.reciprocal_approx_fast(out=rstd_f, in_=srt)
                rstd = work.tile([HC, 1024], f32r, tag="rstd", name="rstd")[:, :fw]
                nc.vector.tensor_copy(rstd, rstd_f)
                bc = ps.tile([P, 1024], f32, tag="A", bufs=3, name="bc")[:, :fw]
                for half in range(fw // 512):
                    hs = slice(half * 512, (half + 1) * 512)
                    nc.tensor.matmul(
                        bc[:, hs], gind[:], rstd[:, hs], start=True, stop=True
                    )
                nc.vector.tensor_mul(dst, raw, bc)

            for b in range(B):
                # ---- KV phase ----
                ctx_sb = ctxp.tile([P, CC, M], f32r, tag="ctx")
                nc.sync.dma_start(
                    ctx_sb[:], ctxt_d[b].rearrange("(o p) m -> p o m", p=P)
                )
                ktn = qk.tile([P, M], f32r, tag="ktn", bufs=2)
                ps_k = ps.tile([P, 1024], f32, tag="A", bufs=3)
                for cc in range(CC):
                    for half in range(2):
                        hs = slice(half * 512, (half + 1) * 512)
                        nc.tensor.matmul(
                            ps_k[:, hs],
                            wk_sb[:, cc],
                            ctx_sb[:, cc, hs],
                            start=(cc == 0),
                            stop=(cc == CC - 1),
                        )
                rms_norm_chunk(ps_k[:], bk_sb, gk_sb, ktn[:], 1024)
                vaug = qk.tile([P, MC, 2 * (D + 1)], f32r, tag="vaug", bufs=2)
                ps_v = ps.tile([P, 1024], f32, tag="A", bufs=3)
                for cc in range(CC):
                    for half in range(2):
                        hs = slice(half * 512, (half + 1) * 512)
                        nc.tensor.matmul(
                            ps_v[:, hs],
                            wv_sb[:, cc],
                            ctx_sb[:, cc, hs],
                            start=(cc == 0),
                            stop=(cc == CC - 1),
                        )
                vt_sb = work.tile([P, M], f32, tag="vt", bufs=1)
                nc.vector.tensor_scalar_add(vt_sb[:], ps_v[:], bv_sb)
                for mc in range(MC):
                    ps_t = ps.tile([P, P], f32, tag="O", bufs=2)
                    nc.tensor.transpose(
                        ps_t[:], vt_sb[:, mc * P : (mc + 1) * P], ident_sb[:]
                    )
                    mcol = mask_sb[:, b * MC + mc : b * MC + mc + 1]
                    nc.vector.tensor_mul(
                        vaug[:, mc, 0:D], ps_t[:, 0:D], mcol.to_broadcast((P, D))
                    )
                    nc.vector.tensor_copy(vaug[:, mc, D : D + 1], mcol)
                    nc.vector.tensor_mul(
                        vaug[:, mc, D + 1 : 2 * D + 1],
                        ps_t[:, D : 2 * D],
                        mcol.to_broadcast((P, D)),
                    )
                    nc.vector.tensor_copy(vaug[:, mc, 2 * D + 1 : 2 * D + 2], mcol)

                # ---- Q projection phase (norms lag one chunk behind) ----
                qtn = qk.tile([P, N], f32r, tag="qtn")
                pending = []
                xt_r = xt_d[b].rearrange("(o p) n -> p o n", p=P)
                for nt in range(NT):
                    nsl = slice(nt * 1024, (nt + 1) * 1024)
                    ps_q = ps.tile([P, 1024], f32, tag="A", bufs=3)
                    for half in range(2):
                        hs = slice(nt * 1024 + half * 512, nt * 1024 + (half + 1) * 512)
                        for cc in range(CC):
                            xt_sb = xp.tile([P, 512], f32r, tag="xt", bufs=8)
                            nc.sync.dma_start(xt_sb[:], xt_r[:, cc, hs])
                            nc.tensor.matmul(
                                ps_q[:, half * 512 : (half + 1) * 512],
                                wq_sb[:, cc],
                                xt_sb[:],
                                start=(cc == 0),
                                stop=(cc == CC - 1),
                            )
                    pending.append((ps_q, nsl))
                    if len(pending) >= 2:
                        pq, pn = pending.pop(0)
                        rms_norm_chunk(pq[:], bq_sb, gq_sb, qtn[:, pn], 1024)
                for pq, pn in pending:
                    rms_norm_chunk(pq[:], bq_sb, gq_sb, qtn[:, pn], 1024)

                # ---- Attention phase (n in chunks of 512) ----
                outtn = qk.tile([P, N], f32r, tag="outtn")
                for nt in range(N // 512):
                    nsl = slice(nt * 512, (nt + 1) * 512)
                    ps_o = [
                        ps.tile([D + 1, 512], f32, tag="O", bufs=2, name=f"ps_o{h}")
                        for h in range(2)
                    ]
                    for mc in range(MC):
                        msl = slice(mc * P, (mc + 1) * P)
                        ps_s = ps.tile([P, 1024], f32, tag="A", bufs=3)
                        for h in range(2):
                            hsl = slice(D * h, D * (h + 1))
                            nc.tensor.matmul(
                                ps_s[:, h * 512 : (h + 1) * 512],
                                ktn[hsl, msl],
                                qtn[hsl, nsl],
                                start=True,
                                stop=True,
                            )
                        pt = ptp.tile([P, 1024], f32r, tag="pt")
                        nc.scalar.activation(pt[:], ps_s[:], AF.Exp)
                        for h in range(2):
                            nc.tensor.matmul(
                                ps_o[h][:],
                                vaug[:, mc, h * (D + 1) : (h + 1) * (D + 1)],
                                pt[:, h * 512 : (h + 1) * 512],
                                start=(mc == 0),
                                stop=(mc == MC - 1),
                            )
                    for h in range(2):
                        denrow = work.tile([1, 512], f32, tag="denrow", bufs=1)
                        nc.vector.tensor_copy(denrow[:], ps_o[h][D : D + 1, :])
                        recrow = work.tile([1, 512], f32, tag="recrow", bufs=1)
                        nc.vector.reciprocal_approx_fast(out=recrow[:], in_=denrow[:])
                        bcn = work.tile([D, 512], f32, tag="bcn", bufs=1)
                        nc.gpsimd.partition_broadcast(bcn[:], recrow[:])
                        nc.vector.scalar_tensor_tensor(
                            out=outtn[D * h : D * (h + 1), nsl],
                            in0=ps_o[h][0:D, :],
                            scalar=1.0,
                            in1=bcn[:],
                            op0=MUL,
                            op1=MUL,
                        )

                # ---- Output projection phase ----
                for tc_ in range(N // P):
                    y_sb = work.tile([P, C], f32, tag="ysb")
                    ps_y = ps.tile([P, 1024], f32, tag="A", bufs=3)
                    for ec in range(2):
                        nc.tensor.matmul(
                            ps_y[:, ec * 512 : (ec + 1) * 512],
                            outtn[:, tc_ * P : (tc_ + 1) * P],
                            wo_sb[:, ec * 512 : (ec + 1) * 512],
                            start=True,
                            stop=True,
                        )
                    nc.scalar.activation(y_sb[:, 0:512], ps_y[:, 0:512], AF.Copy)
                    nc.vector.tensor_copy(y_sb[:, 512:1024], ps_y[:, 512:1024])
                    nc.sync.dma_start(y_d[b, tc_ * P : (tc_ + 1) * P, :], y_sb[:])

    nc.compile()
    _CACHE["nc"] = nc
    return nc


def _make_in_maps(x, context, context_mask, Wq, bq, Wkv, bkv, gq, gk, Wo, bo):
    f32 = np.float32
    xt = np.ascontiguousarray(np.transpose(x, (0, 2, 1)), dtype=f32)
    ctxt = np.ascontiguousarray(np.transpose(context, (0, 2, 1)), dtype=f32)
    # maskf[p, b*MC + mc] = mask[b, mc*128 + p]
    maskf = np.ascontiguousarray(
        np.transpose(
            np.asarray(context_mask, dtype=f32).reshape(B, MC, P), (2, 0, 1)
        ).reshape(P, B * MC)
    )
    ident = np.eye(P, dtype=f32)
    ind2 = np.zeros((P, HC), dtype=f32)
    for h in range(HC):
        ind2[D * h : D * (h + 1), h] = 1.0

    in_maps = []
    for c in range(8):
        hs = slice(P * c, P * (c + 1))
        gq_c = np.zeros((HC, P), dtype=f32)
        gk_c = np.zeros((HC, P), dtype=f32)
        for h in range(HC):
            gq_c[h, D * h : D * (h + 1)] = gq[HC * c + h] * (1.0 / np.sqrt(D))
            gk_c[h, D * h : D * (h + 1)] = gk[HC * c + h]
        in_maps.append(
            {
                "xt": xt,
                "ctxt": ctxt,
                "wqt": np.ascontiguousarray(Wq[hs].T, dtype=f32),
                "wkt": np.ascontiguousarray(Wkv[hs].T, dtype=f32),
                "wvt": np.ascontiguousarray(Wkv[C + P * c : C + P * (c + 1)].T, dtype=f32),
                "wot": np.ascontiguousarray(Wo[:, hs].T, dtype=f32),
                "bq": np.asarray(bq[hs], dtype=f32).reshape(P, 1),
                "bk": np.asarray(bkv[hs], dtype=f32).reshape(P, 1),
                "bv": np.asarray(bkv[C + P * c : C + P * (c + 1)], dtype=f32).reshape(P, 1),
                "gq": gq_c,
                "gk": gk_c,
                "ind2": ind2,
                "ident": ident,
                "maskf": maskf,
            }
        )
    return in_maps


def _run(in_maps, **spmd_kwargs):
    from concourse import bass_utils

    nc = _build()
    return bass_utils.run_bass_kernel_spmd(
        nc, in_maps, core_ids=list(range(8)), **spmd_kwargs
    )


def kernel(x, context, context_mask, Wq, bq, Wkv, bkv, gq, gk, Wo, bo):
    in_maps = _make_in_maps(
        x, context, context_mask, Wq, bq, Wkv, bkv, gq, gk, Wo, bo
    )
    res = _run(in_maps)
    y = np.zeros((B, N, C), dtype=np.float64)
    for c in range(8):
        y += res.results[c]["y"]
    y += np.asarray(bo, dtype=np.float64)
    return y.astype(np.float32)
